# revision 27
# baseline (speedup 1.0000x reference)
"""Trainium2 Bass kernel for nn_BilinAndFwdComboVecComp.

Math (B=8, S=256, C=256, V=64):
  final[b,s,z,k] = tanh( sum_ij ctx[b,s,i] ctx[b,z,j] W'[i,j,k] + A[b,z,k] + Bt[b,s,k] )
where
  W'[i,j,k] = W[i,j,k] + (i==j) * linmul_w[k,i]          (folds the `mul` branch)
  A[b,z,k]  = ctx[b] @ (lin1_w+lindiff_w).T + (lin1_b + bias + linmul_b + lindiff_b
              + lin2_b)   (all (s,z)-constant biases ride the z-side term)
  Bt[b,s,k] = ctx[b] @ (lin2_w-lindiff_w).T              (bias folded into A)

Sharding: V split across the 8 cores (KV=8 k-planes per core). Per core:

  phase 1 (per batch-pair p): tmp2[i,(h,kk,z)] = sum_j W'[i,j,k] ctx[b,z,j]
      - pairs processed as DUOS so each wt stationary serves 2 matmuls
      - psum drain on DVE is a tensor_scalar_add folding M2=(lin2_w-lindiff_w).T
        into tmp2, which makes phase 2's contraction produce the Bt term for free
  phase 2 (TRANSPOSED, per b,kk,zc): psum[z,s] = sum_i tmp2[i,kk,z] ctx[b,s,i]
      - ACT applies tanh with per-partition bias = A[b,z,k] (fp32), so the A
        term costs zero PE work -> NO fold matmuls at all

  The 16 phase-1 kk-blocks and 16 phase-2 chunks run as one flat pipeline
  (chunk j emitted after block j+2), 1:1 in PE rows, so the PE streams
  continuously while DVE drains and ACT tanh trail concurrently.

Inputs are host-preformatted into exact SBUF images (1 descriptor/partition
DMAs) and fanned out need-first across all 4 DMA queues; outputs stream as
zc-split [128,512B] DMAs to keep the tail short. All matmuls fp16.
"""

import numpy as np

B, S, C, V = 8, 256, 256, 64
NCORES = 8
KV = V // NCORES  # k-planes per core


def _host_prep(ctx, W, bias, lin1_w, lin1_b, lin2_w, lin2_b,
               linmul_w, linmul_b, lindiff_w, lindiff_b):
    f = np.float32
    ctx = np.asarray(ctx, f)
    Wp = np.array(W, f)
    Wp[np.arange(C), np.arange(C), :] += np.asarray(linmul_w, f).T

    A = ctx @ (np.asarray(lin1_w, f) + np.asarray(lindiff_w, f)).T \
        + (np.asarray(lin1_b, f) + np.asarray(bias, f)
           + np.asarray(linmul_b, f) + np.asarray(lindiff_b, f)
           + np.asarray(lin2_b, f))  # [B, S, V]
    M2 = (np.asarray(lin2_w, f) - np.asarray(lindiff_w, f)).T  # [C, V]

    # ctx2[jc, c, b*S+z] = ctx[b, z, jc*128+c]  (exact SBUF image)
    ctx2 = np.ascontiguousarray(
        ctx.transpose(2, 0, 1).reshape(2, 128, B * S)).astype(np.float16)

    per_core = []
    for c in range(NCORES):
        ks = slice(c * KV, (c + 1) * KV)
        # wt2[jc, j, kk*C+i] = W'[i, jc*128+j, k0+kk]
        wt2 = np.ascontiguousarray(
            Wp[:, :, ks].transpose(1, 2, 0)        # [j, kk, i]
            .reshape(C, KV * C).reshape(2, 128, KV * C)).astype(np.float16)
        # A_img[zp, zc*64 + b*8 + kk] = A[b, zc*128+zp, k0+kk]
        A_img = np.ascontiguousarray(
            A[:, :, ks].transpose(1, 0, 2)         # [z, b, kk]
            .reshape(2, 128, B * KV).transpose(1, 0, 2)
            .reshape(128, 2 * B * KV)).astype(f)
        # M2_img[ip, ch*8 + kk] = M2[ch*128+ip, k0+kk]
        M2_img = np.ascontiguousarray(
            M2[:, ks].reshape(2, 128, KV).transpose(1, 0, 2)
            .reshape(128, 2 * KV)).astype(f)
        per_core.append({"ctx2": ctx2, "wt2": wt2, "A": A_img, "M2": M2_img})
    return per_core


def _build_program():
    import concourse.tile as tile
    import concourse.mybir as mybir
    from concourse import bacc
    from contextlib import ExitStack

    f32 = mybir.dt.float32
    f16 = mybir.dt.float16
    TANH = mybir.ActivationFunctionType.Tanh

    nc = bacc.Bacc("TRN2", target_bir_lowering=False, debug=False)
    ctx2_d = nc.dram_tensor("ctx2", [2, 128, B * S], f16, kind="ExternalInput").ap()
    wt2_d = nc.dram_tensor("wt2", [2, 128, KV * C], f16, kind="ExternalInput").ap()
    A_dram = nc.dram_tensor("A", [128, 2 * B * KV], f32, kind="ExternalInput").ap()
    M2_dram = nc.dram_tensor("M2", [128, 2 * KV], f32, kind="ExternalInput").ap()
    # out scratch: (b, kk, zc, zp, s); host reshapes/transposes
    out_d = nc.dram_tensor("out", [B, KV, 2, 128, S], f16, kind="ExternalOutput").ap()

    with tile.TileContext(nc) as tc, ExitStack() as es:
        inp_pool = es.enter_context(tc.tile_pool(name="inpp", bufs=1))
        wt_pool = ctx_pool = small_pool = inp_pool
        tmp2_pool = es.enter_context(tc.tile_pool(name="tmp2p", bufs=1))
        ot_pool = es.enter_context(tc.tile_pool(name="otp", bufs=4))
        ps1_pool = es.enter_context(tc.tile_pool(name="ps1", bufs=4, space="PSUM"))
        ps2_pool = es.enter_context(tc.tile_pool(name="ps2", bufs=4, space="PSUM"))

        # ---- input DMAs: need-first order, fanned across all 4 queues ----
        ctx_sb = [ctx_pool.tile([128, B * S], f16, name=f"ctx_{jc}")
                  for jc in range(2)]
        wt_sb = [wt_pool.tile([128, KV * C], f16, name=f"wt_{jc}")
                 for jc in range(2)]
        M2_sb = small_pool.tile([128, 2 * KV], f32, name="M2_sb")
        A_sb = small_pool.tile([128, 2 * B * KV], f32, name="A_sb")
        # need-first fan-out. Small shared deps (M2, A) go FIRST on their
        # queue: Tile coalesces DMA-completion waits by queue position, so a
        # consumer effectively waits for every DMA up to the one it needs.
        H = B * S // 2
        # Coalescing-proof layout: Tile may round a consumer's DMA-wait up to
        # its whole queue, so each queue's TOTAL must land before the
        # earliest need of its last item. sync/gpsimd carry only the
        # early-needed set; all late-needed slices ride the scalar queue.
        nc.sync.dma_start(M2_sb[:], M2_dram)
        nc.gpsimd.dma_start(A_sb[:], A_dram)
        nc.sync.dma_start(wt_sb[0][:, 0:4 * C], wt2_d[0][:, 0:4 * C])
        nc.gpsimd.dma_start(wt_sb[1][:, 0:4 * C], wt2_d[1][:, 0:4 * C])
        nc.sync.dma_start(ctx_sb[0][:, 0:H], ctx2_d[0][:, 0:H])
        nc.gpsimd.dma_start(ctx_sb[1][:, 0:H], ctx2_d[1][:, 0:H])
        nc.scalar.dma_start(wt_sb[0][:, 4 * C:], wt2_d[0][:, 4 * C:])
        nc.scalar.dma_start(wt_sb[1][:, 4 * C:], wt2_d[1][:, 4 * C:])
        nc.scalar.dma_start(ctx_sb[0][:, H:], ctx2_d[0][:, H:])
        nc.scalar.dma_start(ctx_sb[1][:, H:], ctx2_d[1][:, H:])

        # ---- warmup: ramp the PE p-state while inputs land; warm tanh table
        wsrc = small_pool.tile([128, 512], f16, name="wsrc")
        nc.vector.memset(wsrc[:], 0.0)
        wdst = small_pool.tile([128, 8], f16, name="wdst")
        # AP bias (not a float) so no const-tensor load lands on the sync
        # queue ahead of the input DMAs
        nc.scalar.activation(wdst[:, 0:1], wsrc[:, 0:1], TANH, bias=wsrc[:, 1:2])
        for _ in range(8):
            # fresh psum tile per matmul: WAW reuse semaphores would
            # otherwise fragment the p-state ramp
            wps = ps2_pool.tile([128, 512], f32, name="ps2")
            nc.tensor.matmul(wps[:], wsrc[:, 0:128], wsrc[:], start=True, stop=True)

        tmp2 = {}  # (pair, ch) -> tile [128, (h, kk, z)]
        for p in range(4):
            for ch in range(2):
                tmp2[p, ch] = tmp2_pool.tile([128, 2 * KV * S], f16,
                                             name=f"tmp2_{p}_{ch}")

        # Visit order: kk0-3 of a pair-duo first, then their kk4-7, so the
        # first 8 blocks depend only on the first wt DMA slice. Chunk g's
        # producer blocks are exactly 2g and 2g+1 under this mapping.
        SEG_PAIR = [0, 1, 0, 1, 2, 3, 2, 3]
        SEG_KK = [0, 0, 4, 4, 0, 0, 4, 4]

        def p1_block(j):
            """Phase-1 block j (j=0..31): 4 MMs, 2 drains."""
            p = SEG_PAIR[j // 4]
            kk = SEG_KK[j // 4] + j % 4
            for ch in range(2):
                ps = ps1_pool.tile([128, 2 * S], f32, name="ps1")
                for jc in range(2):
                    lhsT = wt_sb[jc][:, kk * C + ch * 128: kk * C + ch * 128 + 128]
                    nc.tensor.matmul(ps[:], lhsT,
                                     ctx_sb[jc][:, 2 * p * S:(2 * p + 2) * S],
                                     start=(jc == 0), stop=(jc == 1))
                dst = tmp2[p, ch][:].rearrange("q (h k z) -> q h k z",
                                               h=2, k=KV)[:, :, kk, :]
                src = ps[:].rearrange("q (h z) -> q h z", h=2)
                nc.vector.tensor_scalar_add(dst, src,
                                            M2_sb[:, ch * KV + kk:
                                                  ch * KV + kk + 1])

        def p2_chunk(g, last=False):
            """Phase-2 chunk g (g=0..15): covers one (pair, kk-pair) for the
            pair's 2 batches: 16 MMs, 8 ACT tanh(+A bias), out DMAs."""
            pair = SEG_PAIR[g // 2]
            kkp = SEG_KK[g // 2] // 2 + g % 2
            for b in (2 * pair, 2 * pair + 1):
                eng = nc.sync if b % 2 == 0 else nc.gpsimd
                ot = ot_pool.tile([128, 4 * S], f16, name="ot")
                otv = ot[:].rearrange("p (k c s) -> p k c s", k=2, c=2)
                for zc in range(2):
                    bank = ps2_pool.tile([128, 512], f32, name="ps2")
                    for kx in range(2):
                        kk = 2 * kkp + kx
                        for st in range(2):
                            lhsT = tmp2[b // 2, st][:].rearrange(
                                "q (h k z) -> q h k z", h=2, k=KV)[
                                :, b % 2, kk, zc * 128:zc * 128 + 128]
                            nc.tensor.matmul(
                                bank[:, kx * 256:kx * 256 + 256], lhsT,
                                ctx_sb[st][:, b * S:(b + 1) * S],
                                start=(st == 0), stop=(st == 1))
                        nc.scalar.activation(
                            otv[:, kx, zc, :],
                            bank[:, kx * 256:kx * 256 + 256], TANH,
                            bias=A_sb[:, zc * 64 + b * KV + kk:
                                      zc * 64 + b * KV + kk + 1])
                    if last:  # split the final DMAs for a shorter tail
                        eng.dma_start(
                            out_d[b, 2 * kkp:2 * kkp + 2, zc].rearrange(
                                "k p s -> p k s"),
                            otv[:, :, zc, :])
                if not last:
                    eng.dma_start(
                        out_d[b, 2 * kkp:2 * kkp + 2].rearrange(
                            "k c p s -> p k c s"),
                        otv[:])

        # ---- flat pipeline: 32 pair-blocks, 16 chunks. Chunk g is emitted
        # immediately after its producer blocks 2g, 2g+1: only their 4
        # drains precede it, so the coalesced drain-wait threshold stays
        # exact and the first chunk starts as soon as B0/B1 are drained ----
        p1_block(0)
        p1_block(1)
        for g in range(16):
            p2_chunk(g, last=(g >= 14))
            if 2 * g + 2 < 32:
                p1_block(2 * g + 2)
            if 2 * g + 3 < 32:
                p1_block(2 * g + 3)

    nc.compile()
    return nc


def _install_profile_hook():
    """Register the NTFF profile hook that the image's boot skipped
    (antenv.axon_hooks shim is missing in this container)."""
    import sys as _sys
    import types as _types
    try:
        import antenv
        if "antenv.axon_hooks" not in _sys.modules:
            m = _types.ModuleType("antenv.axon_hooks")
            _h = [None]
            m.set_axon_ntff_profile_hook = lambda h: _h.__setitem__(0, h)
            m.get_axon_ntff_profile_hook = lambda: _h[0]
            _sys.modules["antenv.axon_hooks"] = m
            antenv.axon_hooks = m
        from antenv.axon_hooks import set_axon_ntff_profile_hook, get_axon_ntff_profile_hook
        if get_axon_ntff_profile_hook() is None:
            from trn_agent_boot.trn_boot import _ntff_profile_via_ctypes
            set_axon_ntff_profile_hook(_ntff_profile_via_ctypes("/opt/axon/libaxon_pjrt.so"))
    except Exception:
        pass


def run(inputs, trace=False, repeats=1):
    """Returns (full_output, BassKernelResults)."""
    from concourse.bass_utils import run_bass_kernel_spmd

    if trace:
        _install_profile_hook()
    per_core = _host_prep(**inputs)
    nc = _build_program()
    import os as _os
    _tc = [int(x) for x in _os.environ.get("KERNEL_TRACE_CORES", "0").split(",")]
    times = []
    res = None
    for _ in range(repeats):
        res = run_bass_kernel_spmd(nc, per_core, list(range(NCORES)), trace=trace,
                                   trace_cores=_tc if trace else None)
        if res.exec_time_ns is not None:
            times.append(res.exec_time_ns)
    if times:
        res.all_exec_times_ns = times
    # per-core scratch is (B, KV, 2, 128, S): reshape to (B, KV, Z, S) then
    # transpose to (B, S, Z, KV); concat k across cores
    out = np.concatenate(
        [res.results[c]["out"].astype(np.float32)
         .reshape(B, KV, S, S).transpose(0, 3, 2, 1)
         for c in range(NCORES)], axis=3)
    out = np.ascontiguousarray(out)
    return out, res


def kernel(**inputs) -> np.ndarray:
    out, _ = run(inputs, trace=False)
    return out


# revision 28
# speedup vs baseline: 1.0167x; 1.0167x over previous
"""Trainium2 Bass kernel for nn_BilinAndFwdComboVecComp.

Math (B=8, S=256, C=256, V=64):
  final[b,s,z,k] = tanh( sum_ij ctx[b,s,i] ctx[b,z,j] W'[i,j,k] + A[b,z,k] + Bt[b,s,k] )
where
  W'[i,j,k] = W[i,j,k] + (i==j) * linmul_w[k,i]          (folds the `mul` branch)
  A[b,z,k]  = ctx[b] @ (lin1_w+lindiff_w).T + (lin1_b + bias + linmul_b + lindiff_b
              + lin2_b)   (all (s,z)-constant biases ride the z-side term)
  Bt[b,s,k] = ctx[b] @ (lin2_w-lindiff_w).T              (bias folded into A)

Sharding: V split across the 8 cores (KV=8 k-planes per core). Per core:

  phase 1 (per batch-pair p): tmp2[i,(h,kk,z)] = sum_j W'[i,j,k] ctx[b,z,j]
      - pairs processed as DUOS so each wt stationary serves 2 matmuls
      - psum drain on DVE is a tensor_scalar_add folding M2=(lin2_w-lindiff_w).T
        into tmp2, which makes phase 2's contraction produce the Bt term for free
  phase 2 (TRANSPOSED, per b,kk,zc): psum[z,s] = sum_i tmp2[i,kk,z] ctx[b,s,i]
      - ACT applies tanh with per-partition bias = A[b,z,k] (fp32), so the A
        term costs zero PE work -> NO fold matmuls at all

  The 16 phase-1 kk-blocks and 16 phase-2 chunks run as one flat pipeline
  (chunk j emitted after block j+2), 1:1 in PE rows, so the PE streams
  continuously while DVE drains and ACT tanh trail concurrently.

Inputs are host-preformatted into exact SBUF images (1 descriptor/partition
DMAs) and fanned out need-first across all 4 DMA queues; outputs stream as
zc-split [128,512B] DMAs to keep the tail short. All matmuls fp16.
"""

import numpy as np

B, S, C, V = 8, 256, 256, 64
NCORES = 8
KV = V // NCORES  # k-planes per core


def _host_prep(ctx, W, bias, lin1_w, lin1_b, lin2_w, lin2_b,
               linmul_w, linmul_b, lindiff_w, lindiff_b):
    f = np.float32
    ctx = np.asarray(ctx, f)
    Wp = np.array(W, f)
    Wp[np.arange(C), np.arange(C), :] += np.asarray(linmul_w, f).T

    A = ctx @ (np.asarray(lin1_w, f) + np.asarray(lindiff_w, f)).T \
        + (np.asarray(lin1_b, f) + np.asarray(bias, f)
           + np.asarray(linmul_b, f) + np.asarray(lindiff_b, f)
           + np.asarray(lin2_b, f))  # [B, S, V]
    M2 = (np.asarray(lin2_w, f) - np.asarray(lindiff_w, f)).T  # [C, V]

    # ctx2[jc, c, b*S+z] = ctx[b, z, jc*128+c]  (exact SBUF image)
    ctx2 = np.ascontiguousarray(
        ctx.transpose(2, 0, 1).reshape(2, 128, B * S)).astype(np.float16)

    per_core = []
    for c in range(NCORES):
        ks = slice(c * KV, (c + 1) * KV)
        # wt2[jc, j, kk*C+i] = W'[i, jc*128+j, k0+kk]
        wt2 = np.ascontiguousarray(
            Wp[:, :, ks].transpose(1, 2, 0)        # [j, kk, i]
            .reshape(C, KV * C).reshape(2, 128, KV * C)).astype(np.float16)
        # A_img[zp, zc*64 + b*8 + kk] = A[b, zc*128+zp, k0+kk]
        A_img = np.ascontiguousarray(
            A[:, :, ks].transpose(1, 0, 2)         # [z, b, kk]
            .reshape(2, 128, B * KV).transpose(1, 0, 2)
            .reshape(128, 2 * B * KV)).astype(f)
        # M2_img[ip, ch*8 + kk] = M2[ch*128+ip, k0+kk]
        M2_img = np.ascontiguousarray(
            M2[:, ks].reshape(2, 128, KV).transpose(1, 0, 2)
            .reshape(128, 2 * KV)).astype(f)
        per_core.append({"ctx2": ctx2, "wt2": wt2, "A": A_img, "M2": M2_img})
    return per_core


def _build_program():
    import concourse.tile as tile
    import concourse.mybir as mybir
    from concourse import bacc
    from contextlib import ExitStack

    f32 = mybir.dt.float32
    f16 = mybir.dt.float16
    TANH = mybir.ActivationFunctionType.Tanh

    nc = bacc.Bacc("TRN2", target_bir_lowering=False, debug=False)
    ctx2_d = nc.dram_tensor("ctx2", [2, 128, B * S], f16, kind="ExternalInput").ap()
    wt2_d = nc.dram_tensor("wt2", [2, 128, KV * C], f16, kind="ExternalInput").ap()
    A_dram = nc.dram_tensor("A", [128, 2 * B * KV], f32, kind="ExternalInput").ap()
    M2_dram = nc.dram_tensor("M2", [128, 2 * KV], f32, kind="ExternalInput").ap()
    # out scratch: (b, kk, zc, zp, s); host reshapes/transposes
    out_d = nc.dram_tensor("out", [B, KV, 2, 128, S], f16, kind="ExternalOutput").ap()

    with tile.TileContext(nc) as tc, ExitStack() as es:
        inp_pool = es.enter_context(tc.tile_pool(name="inpp", bufs=1))
        wt_pool = ctx_pool = small_pool = inp_pool
        tmp2_pool = es.enter_context(tc.tile_pool(name="tmp2p", bufs=1))
        ot_pool = es.enter_context(tc.tile_pool(name="otp", bufs=4))
        ps1_pool = es.enter_context(tc.tile_pool(name="ps1", bufs=4, space="PSUM"))
        ps2_pool = es.enter_context(tc.tile_pool(name="ps2", bufs=4, space="PSUM"))

        # ---- input DMAs: need-first order, fanned across all 4 queues ----
        ctx_sb = [ctx_pool.tile([128, B * S], f16, name=f"ctx_{jc}")
                  for jc in range(2)]
        wt_sb = [wt_pool.tile([128, KV * C], f16, name=f"wt_{jc}")
                 for jc in range(2)]
        M2_sb = small_pool.tile([128, 2 * KV], f32, name="M2_sb")
        A_sb = small_pool.tile([128, 2 * B * KV], f32, name="A_sb")
        # need-first fan-out. Small shared deps (M2, A) go FIRST on their
        # queue: Tile coalesces DMA-completion waits by queue position, so a
        # consumer effectively waits for every DMA up to the one it needs.
        H = B * S // 2
        nc.sync.dma_start(wt_sb[0][:, 0:4 * C], wt2_d[0][:, 0:4 * C])
        nc.gpsimd.dma_start(wt_sb[1][:, 0:4 * C], wt2_d[1][:, 0:4 * C])
        nc.scalar.dma_start(M2_sb[:], M2_dram)
        nc.scalar.dma_start(A_sb[:], A_dram)
        nc.sync.dma_start(ctx_sb[0][:, 0:H], ctx2_d[0][:, 0:H])
        nc.gpsimd.dma_start(ctx_sb[1][:, 0:H], ctx2_d[1][:, 0:H])
        nc.scalar.dma_start(wt_sb[0][:, 4 * C:], wt2_d[0][:, 4 * C:])
        nc.scalar.dma_start(wt_sb[1][:, 4 * C:], wt2_d[1][:, 4 * C:])
        nc.sync.dma_start(ctx_sb[0][:, H:], ctx2_d[0][:, H:])
        nc.gpsimd.dma_start(ctx_sb[1][:, H:], ctx2_d[1][:, H:])

        # ---- warmup: ramp the PE p-state while inputs land; warm tanh table
        wsrc = small_pool.tile([128, 512], f16, name="wsrc")
        nc.vector.memset(wsrc[:], 0.0)
        wdst = small_pool.tile([128, 8], f16, name="wdst")
        # AP bias (not a float) so no const-tensor load lands on the sync
        # queue ahead of the input DMAs
        nc.scalar.activation(wdst[:, 0:1], wsrc[:, 0:1], TANH, bias=wsrc[:, 1:2])
        for _ in range(8):
            # fresh psum tile per matmul: WAW reuse semaphores would
            # otherwise fragment the p-state ramp
            wps = ps2_pool.tile([128, 512], f32, name="ps2")
            nc.tensor.matmul(wps[:], wsrc[:, 0:128], wsrc[:], start=True, stop=True)

        tmp2 = {}  # (pair, ch) -> tile [128, (h, kk, z)]
        for p in range(4):
            for ch in range(2):
                tmp2[p, ch] = tmp2_pool.tile([128, 2 * KV * S], f16,
                                             name=f"tmp2_{p}_{ch}")

        # Visit order: kk0-3 of a pair-duo first, then their kk4-7, so the
        # first 8 blocks depend only on the first wt DMA slice. Chunk g's
        # producer blocks are exactly 2g and 2g+1 under this mapping.
        SEG_PAIR = [0, 1, 0, 1, 2, 3, 2, 3]
        SEG_KK = [0, 0, 4, 4, 0, 0, 4, 4]

        def p1_block(j):
            """Phase-1 block j (j=0..31): 4 MMs, 2 drains."""
            p = SEG_PAIR[j // 4]
            kk = SEG_KK[j // 4] + j % 4
            for ch in range(2):
                ps = ps1_pool.tile([128, 2 * S], f32, name="ps1")
                for jc in range(2):
                    lhsT = wt_sb[jc][:, kk * C + ch * 128: kk * C + ch * 128 + 128]
                    nc.tensor.matmul(ps[:], lhsT,
                                     ctx_sb[jc][:, 2 * p * S:(2 * p + 2) * S],
                                     start=(jc == 0), stop=(jc == 1))
                dst = tmp2[p, ch][:].rearrange("q (h k z) -> q h k z",
                                               h=2, k=KV)[:, :, kk, :]
                src = ps[:].rearrange("q (h z) -> q h z", h=2)
                nc.vector.tensor_scalar_add(dst, src,
                                            M2_sb[:, ch * KV + kk:
                                                  ch * KV + kk + 1])

        def p2_chunk(g, last=False):
            """Phase-2 chunk g (g=0..15): covers one (pair, kk-pair) for the
            pair's 2 batches: 16 MMs, 8 ACT tanh(+A bias), out DMAs."""
            pair = SEG_PAIR[g // 2]
            kkp = SEG_KK[g // 2] // 2 + g % 2
            for b in (2 * pair, 2 * pair + 1):
                eng = nc.sync if b % 2 == 0 else nc.gpsimd
                ot = ot_pool.tile([128, 4 * S], f16, name="ot")
                otv = ot[:].rearrange("p (k c s) -> p k c s", k=2, c=2)
                for zc in range(2):
                    bank = ps2_pool.tile([128, 512], f32, name="ps2")
                    for kx in range(2):
                        kk = 2 * kkp + kx
                        for st in range(2):
                            lhsT = tmp2[b // 2, st][:].rearrange(
                                "q (h k z) -> q h k z", h=2, k=KV)[
                                :, b % 2, kk, zc * 128:zc * 128 + 128]
                            nc.tensor.matmul(
                                bank[:, kx * 256:kx * 256 + 256], lhsT,
                                ctx_sb[st][:, b * S:(b + 1) * S],
                                start=(st == 0), stop=(st == 1))
                        nc.scalar.activation(
                            otv[:, kx, zc, :],
                            bank[:, kx * 256:kx * 256 + 256], TANH,
                            bias=A_sb[:, zc * 64 + b * KV + kk:
                                      zc * 64 + b * KV + kk + 1])
                    if last:  # split the final DMAs for a shorter tail
                        eng.dma_start(
                            out_d[b, 2 * kkp:2 * kkp + 2, zc].rearrange(
                                "k p s -> p k s"),
                            otv[:, :, zc, :])
                if not last:
                    eng.dma_start(
                        out_d[b, 2 * kkp:2 * kkp + 2].rearrange(
                            "k c p s -> p k c s"),
                        otv[:])

        # ---- flat pipeline: 32 pair-blocks, 16 chunks. Chunk g is emitted
        # immediately after its producer blocks 2g, 2g+1: only their 4
        # drains precede it, so the coalesced drain-wait threshold stays
        # exact and the first chunk starts as soon as B0/B1 are drained ----
        p1_block(0)
        p1_block(1)
        for g in range(16):
            p2_chunk(g, last=(g >= 14))
            if 2 * g + 2 < 32:
                p1_block(2 * g + 2)
            if 2 * g + 3 < 32:
                p1_block(2 * g + 3)

    nc.compile()
    return nc


def _install_profile_hook():
    """Register the NTFF profile hook that the image's boot skipped
    (antenv.axon_hooks shim is missing in this container)."""
    import sys as _sys
    import types as _types
    try:
        import antenv
        if "antenv.axon_hooks" not in _sys.modules:
            m = _types.ModuleType("antenv.axon_hooks")
            _h = [None]
            m.set_axon_ntff_profile_hook = lambda h: _h.__setitem__(0, h)
            m.get_axon_ntff_profile_hook = lambda: _h[0]
            _sys.modules["antenv.axon_hooks"] = m
            antenv.axon_hooks = m
        from antenv.axon_hooks import set_axon_ntff_profile_hook, get_axon_ntff_profile_hook
        if get_axon_ntff_profile_hook() is None:
            from trn_agent_boot.trn_boot import _ntff_profile_via_ctypes
            set_axon_ntff_profile_hook(_ntff_profile_via_ctypes("/opt/axon/libaxon_pjrt.so"))
    except Exception:
        pass


def run(inputs, trace=False, repeats=1):
    """Returns (full_output, BassKernelResults)."""
    from concourse.bass_utils import run_bass_kernel_spmd

    if trace:
        _install_profile_hook()
    per_core = _host_prep(**inputs)
    nc = _build_program()
    import os as _os
    _tc = [int(x) for x in _os.environ.get("KERNEL_TRACE_CORES", "0").split(",")]
    times = []
    res = None
    for _ in range(repeats):
        res = run_bass_kernel_spmd(nc, per_core, list(range(NCORES)), trace=trace,
                                   trace_cores=_tc if trace else None)
        if res.exec_time_ns is not None:
            times.append(res.exec_time_ns)
    if times:
        res.all_exec_times_ns = times
    # per-core scratch is (B, KV, 2, 128, S): reshape to (B, KV, Z, S) then
    # transpose to (B, S, Z, KV); concat k across cores
    out = np.concatenate(
        [res.results[c]["out"].astype(np.float32)
         .reshape(B, KV, S, S).transpose(0, 3, 2, 1)
         for c in range(NCORES)], axis=3)
    out = np.ascontiguousarray(out)
    return out, res


def kernel(**inputs) -> np.ndarray:
    out, _ = run(inputs, trace=False)
    return out


# revision 29
# speedup vs baseline: 1.0228x; 1.0060x over previous
"""Trainium2 Bass kernel for nn_BilinAndFwdComboVecComp.

Math (B=8, S=256, C=256, V=64):
  final[b,s,z,k] = tanh( sum_ij ctx[b,s,i] ctx[b,z,j] W'[i,j,k] + A[b,z,k] + Bt[b,s,k] )
where
  W'[i,j,k] = W[i,j,k] + (i==j) * linmul_w[k,i]          (folds the `mul` branch)
  A[b,z,k]  = ctx[b] @ (lin1_w+lindiff_w).T + (lin1_b + bias + linmul_b + lindiff_b
              + lin2_b)   (all (s,z)-constant biases ride the z-side term)
  Bt[b,s,k] = ctx[b] @ (lin2_w-lindiff_w).T              (bias folded into A)

Sharding: V split across the 8 cores (KV=8 k-planes per core). Per core:

  phase 1 (per batch-pair p): tmp2[i,(h,kk,z)] = sum_j W'[i,j,k] ctx[b,z,j]
      - pairs processed as DUOS so each wt stationary serves 2 matmuls
      - psum drain on DVE is a tensor_scalar_add folding M2=(lin2_w-lindiff_w).T
        into tmp2, which makes phase 2's contraction produce the Bt term for free
  phase 2 (TRANSPOSED, per b,kk,zc): psum[z,s] = sum_i tmp2[i,kk,z] ctx[b,s,i]
      - ACT applies tanh with per-partition bias = A[b,z,k] (fp32), so the A
        term costs zero PE work -> NO fold matmuls at all

  The 16 phase-1 kk-blocks and 16 phase-2 chunks run as one flat pipeline
  (chunk j emitted after block j+2), 1:1 in PE rows, so the PE streams
  continuously while DVE drains and ACT tanh trail concurrently.

Inputs are host-preformatted into exact SBUF images (1 descriptor/partition
DMAs) and fanned out need-first across all 4 DMA queues; outputs stream as
zc-split [128,512B] DMAs to keep the tail short. All matmuls fp16.
"""

import numpy as np

B, S, C, V = 8, 256, 256, 64
NCORES = 8
KV = V // NCORES  # k-planes per core


def _host_prep(ctx, W, bias, lin1_w, lin1_b, lin2_w, lin2_b,
               linmul_w, linmul_b, lindiff_w, lindiff_b):
    f = np.float32
    ctx = np.asarray(ctx, f)
    Wp = np.array(W, f)
    Wp[np.arange(C), np.arange(C), :] += np.asarray(linmul_w, f).T

    A = ctx @ (np.asarray(lin1_w, f) + np.asarray(lindiff_w, f)).T \
        + (np.asarray(lin1_b, f) + np.asarray(bias, f)
           + np.asarray(linmul_b, f) + np.asarray(lindiff_b, f)
           + np.asarray(lin2_b, f))  # [B, S, V]
    M2 = (np.asarray(lin2_w, f) - np.asarray(lindiff_w, f)).T  # [C, V]

    # ctx2[jc, c, b*S+z] = ctx[b, z, jc*128+c]  (exact SBUF image)
    ctx2 = np.ascontiguousarray(
        ctx.transpose(2, 0, 1).reshape(2, 128, B * S)).astype(np.float16)

    per_core = []
    for c in range(NCORES):
        ks = slice(c * KV, (c + 1) * KV)
        # wt2[jc, j, kk*C+i] = W'[i, jc*128+j, k0+kk]
        wt2 = np.ascontiguousarray(
            Wp[:, :, ks].transpose(1, 2, 0)        # [j, kk, i]
            .reshape(C, KV * C).reshape(2, 128, KV * C)).astype(np.float16)
        # A_img[zp, zc*64 + b*8 + kk] = A[b, zc*128+zp, k0+kk]
        A_img = np.ascontiguousarray(
            A[:, :, ks].transpose(1, 0, 2)         # [z, b, kk]
            .reshape(2, 128, B * KV).transpose(1, 0, 2)
            .reshape(128, 2 * B * KV)).astype(f)
        # M2_img[ip, ch*8 + kk] = M2[ch*128+ip, k0+kk]
        M2_img = np.ascontiguousarray(
            M2[:, ks].reshape(2, 128, KV).transpose(1, 0, 2)
            .reshape(128, 2 * KV)).astype(f)
        per_core.append({"ctx2": ctx2, "wt2": wt2, "A": A_img, "M2": M2_img})
    return per_core


def _build_program():
    import concourse.tile as tile
    import concourse.mybir as mybir
    from concourse import bacc
    from contextlib import ExitStack

    f32 = mybir.dt.float32
    f16 = mybir.dt.float16
    TANH = mybir.ActivationFunctionType.Tanh

    nc = bacc.Bacc("TRN2", target_bir_lowering=False, debug=False)
    ctx2_d = nc.dram_tensor("ctx2", [2, 128, B * S], f16, kind="ExternalInput").ap()
    wt2_d = nc.dram_tensor("wt2", [2, 128, KV * C], f16, kind="ExternalInput").ap()
    A_dram = nc.dram_tensor("A", [128, 2 * B * KV], f32, kind="ExternalInput").ap()
    M2_dram = nc.dram_tensor("M2", [128, 2 * KV], f32, kind="ExternalInput").ap()
    # out scratch: (b, kk, zc, zp, s); host reshapes/transposes
    out_d = nc.dram_tensor("out", [B, KV, 2, 128, S], f16, kind="ExternalOutput").ap()

    with tile.TileContext(nc) as tc, ExitStack() as es:
        inp_pool = es.enter_context(tc.tile_pool(name="inpp", bufs=1))
        wt_pool = ctx_pool = small_pool = inp_pool
        tmp2_pool = es.enter_context(tc.tile_pool(name="tmp2p", bufs=1))
        ot_pool = es.enter_context(tc.tile_pool(name="otp", bufs=4))
        ps1_pool = es.enter_context(tc.tile_pool(name="ps1", bufs=3, space="PSUM"))
        ps2_pool = es.enter_context(tc.tile_pool(name="ps2", bufs=5, space="PSUM"))

        # ---- input DMAs: need-first order, fanned across all 4 queues ----
        ctx_sb = [ctx_pool.tile([128, B * S], f16, name=f"ctx_{jc}")
                  for jc in range(2)]
        wt_sb = [wt_pool.tile([128, KV * C], f16, name=f"wt_{jc}")
                 for jc in range(2)]
        M2_sb = small_pool.tile([128, 2 * KV], f32, name="M2_sb")
        A_sb = small_pool.tile([128, 2 * B * KV], f32, name="A_sb")
        # need-first fan-out. Small shared deps (M2, A) go FIRST on their
        # queue: Tile coalesces DMA-completion waits by queue position, so a
        # consumer effectively waits for every DMA up to the one it needs.
        H = B * S // 2
        nc.sync.dma_start(wt_sb[0][:, 0:4 * C], wt2_d[0][:, 0:4 * C])
        nc.gpsimd.dma_start(wt_sb[1][:, 0:4 * C], wt2_d[1][:, 0:4 * C])
        nc.scalar.dma_start(M2_sb[:], M2_dram)
        nc.scalar.dma_start(A_sb[:], A_dram)
        nc.sync.dma_start(ctx_sb[0][:, 0:H], ctx2_d[0][:, 0:H])
        nc.gpsimd.dma_start(ctx_sb[1][:, 0:H], ctx2_d[1][:, 0:H])
        nc.scalar.dma_start(wt_sb[0][:, 4 * C:], wt2_d[0][:, 4 * C:])
        nc.scalar.dma_start(wt_sb[1][:, 4 * C:], wt2_d[1][:, 4 * C:])
        nc.sync.dma_start(ctx_sb[0][:, H:], ctx2_d[0][:, H:])
        nc.gpsimd.dma_start(ctx_sb[1][:, H:], ctx2_d[1][:, H:])

        # ---- warmup: ramp the PE p-state while inputs land; warm tanh table
        wsrc = small_pool.tile([128, 512], f16, name="wsrc")
        nc.vector.memset(wsrc[:], 0.0)
        wdst = small_pool.tile([128, 8], f16, name="wdst")
        # AP bias (not a float) so no const-tensor load lands on the sync
        # queue ahead of the input DMAs
        nc.scalar.activation(wdst[:, 0:1], wsrc[:, 0:1], TANH, bias=wsrc[:, 1:2])
        for _ in range(8):
            # fresh psum tile per matmul: WAW reuse semaphores would
            # otherwise fragment the p-state ramp
            wps = ps2_pool.tile([128, 512], f32, name="ps2")
            nc.tensor.matmul(wps[:], wsrc[:, 0:128], wsrc[:], start=True, stop=True)

        tmp2 = {}  # (pair, ch) -> tile [128, (h, kk, z)]
        for p in range(4):
            for ch in range(2):
                tmp2[p, ch] = tmp2_pool.tile([128, 2 * KV * S], f16,
                                             name=f"tmp2_{p}_{ch}")

        # Visit order: kk0-3 of a pair-duo first, then their kk4-7, so the
        # first 8 blocks depend only on the first wt DMA slice. Chunk g's
        # producer blocks are exactly 2g and 2g+1 under this mapping.
        SEG_PAIR = [0, 1, 0, 1, 2, 3, 2, 3]
        SEG_KK = [0, 0, 4, 4, 0, 0, 4, 4]

        def p1_block(j):
            """Phase-1 block j (j=0..31): 4 MMs, 2 drains."""
            p = SEG_PAIR[j // 4]
            kk = SEG_KK[j // 4] + j % 4
            for ch in range(2):
                ps = ps1_pool.tile([128, 2 * S], f32, name="ps1")
                for jc in range(2):
                    lhsT = wt_sb[jc][:, kk * C + ch * 128: kk * C + ch * 128 + 128]
                    nc.tensor.matmul(ps[:], lhsT,
                                     ctx_sb[jc][:, 2 * p * S:(2 * p + 2) * S],
                                     start=(jc == 0), stop=(jc == 1))
                dst = tmp2[p, ch][:].rearrange("q (h k z) -> q h k z",
                                               h=2, k=KV)[:, :, kk, :]
                src = ps[:].rearrange("q (h z) -> q h z", h=2)
                nc.vector.tensor_scalar_add(dst, src,
                                            M2_sb[:, ch * KV + kk:
                                                  ch * KV + kk + 1])

        def p2_chunk(g, last=False):
            """Phase-2 chunk g (g=0..15): covers one (pair, kk-pair) for the
            pair's 2 batches: 16 MMs, 8 ACT tanh(+A bias), out DMAs."""
            pair = SEG_PAIR[g // 2]
            kkp = SEG_KK[g // 2] // 2 + g % 2
            for b in (2 * pair, 2 * pair + 1):
                eng = nc.sync if b % 2 == 0 else nc.gpsimd
                ot = ot_pool.tile([128, 4 * S], f16, name="ot")
                otv = ot[:].rearrange("p (k c s) -> p k c s", k=2, c=2)
                for zc in range(2):
                    bank = ps2_pool.tile([128, 512], f32, name="ps2")
                    for kx in range(2):
                        kk = 2 * kkp + kx
                        for st in range(2):
                            lhsT = tmp2[b // 2, st][:].rearrange(
                                "q (h k z) -> q h k z", h=2, k=KV)[
                                :, b % 2, kk, zc * 128:zc * 128 + 128]
                            nc.tensor.matmul(
                                bank[:, kx * 256:kx * 256 + 256], lhsT,
                                ctx_sb[st][:, b * S:(b + 1) * S],
                                start=(st == 0), stop=(st == 1))
                        nc.scalar.activation(
                            otv[:, kx, zc, :],
                            bank[:, kx * 256:kx * 256 + 256], TANH,
                            bias=A_sb[:, zc * 64 + b * KV + kk:
                                      zc * 64 + b * KV + kk + 1])
                    if last:  # split the final DMAs for a shorter tail
                        eng.dma_start(
                            out_d[b, 2 * kkp:2 * kkp + 2, zc].rearrange(
                                "k p s -> p k s"),
                            otv[:, :, zc, :])
                if not last:
                    eng.dma_start(
                        out_d[b, 2 * kkp:2 * kkp + 2].rearrange(
                            "k c p s -> p k c s"),
                        otv[:])

        # ---- flat pipeline: 32 pair-blocks, 16 chunks. Chunk g is emitted
        # immediately after its producer blocks 2g, 2g+1: only their 4
        # drains precede it, so the coalesced drain-wait threshold stays
        # exact and the first chunk starts as soon as B0/B1 are drained ----
        p1_block(0)
        p1_block(1)
        for g in range(16):
            p2_chunk(g, last=(g >= 14))
            if 2 * g + 2 < 32:
                p1_block(2 * g + 2)
            if 2 * g + 3 < 32:
                p1_block(2 * g + 3)

    nc.compile()
    return nc


def _install_profile_hook():
    """Register the NTFF profile hook that the image's boot skipped
    (antenv.axon_hooks shim is missing in this container)."""
    import sys as _sys
    import types as _types
    try:
        import antenv
        if "antenv.axon_hooks" not in _sys.modules:
            m = _types.ModuleType("antenv.axon_hooks")
            _h = [None]
            m.set_axon_ntff_profile_hook = lambda h: _h.__setitem__(0, h)
            m.get_axon_ntff_profile_hook = lambda: _h[0]
            _sys.modules["antenv.axon_hooks"] = m
            antenv.axon_hooks = m
        from antenv.axon_hooks import set_axon_ntff_profile_hook, get_axon_ntff_profile_hook
        if get_axon_ntff_profile_hook() is None:
            from trn_agent_boot.trn_boot import _ntff_profile_via_ctypes
            set_axon_ntff_profile_hook(_ntff_profile_via_ctypes("/opt/axon/libaxon_pjrt.so"))
    except Exception:
        pass


def run(inputs, trace=False, repeats=1):
    """Returns (full_output, BassKernelResults)."""
    from concourse.bass_utils import run_bass_kernel_spmd

    if trace:
        _install_profile_hook()
    per_core = _host_prep(**inputs)
    nc = _build_program()
    import os as _os
    _tc = [int(x) for x in _os.environ.get("KERNEL_TRACE_CORES", "0").split(",")]
    times = []
    res = None
    for _ in range(repeats):
        res = run_bass_kernel_spmd(nc, per_core, list(range(NCORES)), trace=trace,
                                   trace_cores=_tc if trace else None)
        if res.exec_time_ns is not None:
            times.append(res.exec_time_ns)
    if times:
        res.all_exec_times_ns = times
    # per-core scratch is (B, KV, 2, 128, S): reshape to (B, KV, Z, S) then
    # transpose to (B, S, Z, KV); concat k across cores
    out = np.concatenate(
        [res.results[c]["out"].astype(np.float32)
         .reshape(B, KV, S, S).transpose(0, 3, 2, 1)
         for c in range(NCORES)], axis=3)
    out = np.ascontiguousarray(out)
    return out, res


def kernel(**inputs) -> np.ndarray:
    out, _ = run(inputs, trace=False)
    return out


# revision 30
# speedup vs baseline: 1.0286x; 1.0057x over previous
"""Trainium2 Bass kernel for nn_BilinAndFwdComboVecComp.

Math (B=8, S=256, C=256, V=64):
  final[b,s,z,k] = tanh( sum_ij ctx[b,s,i] ctx[b,z,j] W'[i,j,k] + A[b,z,k] + Bt[b,s,k] )
where
  W'[i,j,k] = W[i,j,k] + (i==j) * linmul_w[k,i]          (folds the `mul` branch)
  A[b,z,k]  = ctx[b] @ (lin1_w+lindiff_w).T + (lin1_b + bias + linmul_b + lindiff_b
              + lin2_b)   (all (s,z)-constant biases ride the z-side term)
  Bt[b,s,k] = ctx[b] @ (lin2_w-lindiff_w).T              (bias folded into A)

Sharding: V split across the 8 cores (KV=8 k-planes per core). Per core:

  phase 1 (per batch-pair p): tmp2[i,(h,kk,z)] = sum_j W'[i,j,k] ctx[b,z,j]
      - pairs processed as DUOS so each wt stationary serves 2 matmuls
      - psum drain on DVE is a tensor_scalar_add folding M2=(lin2_w-lindiff_w).T
        into tmp2, which makes phase 2's contraction produce the Bt term for free
  phase 2 (TRANSPOSED, per b,kk,zc): psum[z,s] = sum_i tmp2[i,kk,z] ctx[b,s,i]
      - ACT applies tanh with per-partition bias = A[b,z,k] (fp32), so the A
        term costs zero PE work -> NO fold matmuls at all

  The 16 phase-1 kk-blocks and 16 phase-2 chunks run as one flat pipeline
  (chunk j emitted after block j+2), 1:1 in PE rows, so the PE streams
  continuously while DVE drains and ACT tanh trail concurrently.

Inputs are host-preformatted into exact SBUF images (1 descriptor/partition
DMAs) and fanned out need-first across all 4 DMA queues; outputs stream as
zc-split [128,512B] DMAs to keep the tail short. All matmuls fp16.
"""

import numpy as np

B, S, C, V = 8, 256, 256, 64
NCORES = 8
KV = V // NCORES  # k-planes per core


def _host_prep(ctx, W, bias, lin1_w, lin1_b, lin2_w, lin2_b,
               linmul_w, linmul_b, lindiff_w, lindiff_b):
    f = np.float32
    ctx = np.asarray(ctx, f)
    Wp = np.array(W, f)
    Wp[np.arange(C), np.arange(C), :] += np.asarray(linmul_w, f).T

    A = ctx @ (np.asarray(lin1_w, f) + np.asarray(lindiff_w, f)).T \
        + (np.asarray(lin1_b, f) + np.asarray(bias, f)
           + np.asarray(linmul_b, f) + np.asarray(lindiff_b, f)
           + np.asarray(lin2_b, f))  # [B, S, V]
    M2 = (np.asarray(lin2_w, f) - np.asarray(lindiff_w, f)).T  # [C, V]

    # ctx2[jc, c, b*S+z] = ctx[b, z, jc*128+c]  (exact SBUF image)
    ctx2 = np.ascontiguousarray(
        ctx.transpose(2, 0, 1).reshape(2, 128, B * S)).astype(np.float16)

    per_core = []
    for c in range(NCORES):
        ks = slice(c * KV, (c + 1) * KV)
        # wt2[jc, j, kk*C+i] = W'[i, jc*128+j, k0+kk]
        wt2 = np.ascontiguousarray(
            Wp[:, :, ks].transpose(1, 2, 0)        # [j, kk, i]
            .reshape(C, KV * C).reshape(2, 128, KV * C)).astype(np.float16)
        # A_img[zp, zc*64 + b*8 + kk] = A[b, zc*128+zp, k0+kk]
        A_img = np.ascontiguousarray(
            A[:, :, ks].transpose(1, 0, 2)         # [z, b, kk]
            .reshape(2, 128, B * KV).transpose(1, 0, 2)
            .reshape(128, 2 * B * KV)).astype(f)
        # M2_img[ip, ch*8 + kk] = M2[ch*128+ip, k0+kk]
        M2_img = np.ascontiguousarray(
            M2[:, ks].reshape(2, 128, KV).transpose(1, 0, 2)
            .reshape(128, 2 * KV)).astype(f)
        per_core.append({"ctx2": ctx2, "wt2": wt2, "A": A_img, "M2": M2_img})
    return per_core


def _build_program():
    import concourse.tile as tile
    import concourse.mybir as mybir
    from concourse import bacc
    from contextlib import ExitStack

    f32 = mybir.dt.float32
    f16 = mybir.dt.float16
    TANH = mybir.ActivationFunctionType.Tanh

    nc = bacc.Bacc("TRN2", target_bir_lowering=False, debug=False)
    ctx2_d = nc.dram_tensor("ctx2", [2, 128, B * S], f16, kind="ExternalInput").ap()
    wt2_d = nc.dram_tensor("wt2", [2, 128, KV * C], f16, kind="ExternalInput").ap()
    A_dram = nc.dram_tensor("A", [128, 2 * B * KV], f32, kind="ExternalInput").ap()
    M2_dram = nc.dram_tensor("M2", [128, 2 * KV], f32, kind="ExternalInput").ap()
    # out scratch: (b, kk, zc, zp, s); host reshapes/transposes
    out_d = nc.dram_tensor("out", [B, KV, 2, 128, S], f16, kind="ExternalOutput").ap()

    with tile.TileContext(nc) as tc, ExitStack() as es:
        inp_pool = es.enter_context(tc.tile_pool(name="inpp", bufs=1))
        wt_pool = ctx_pool = small_pool = inp_pool
        tmp2_pool = es.enter_context(tc.tile_pool(name="tmp2p", bufs=1))
        ot_pool = es.enter_context(tc.tile_pool(name="otp", bufs=4))
        ps1_pool = es.enter_context(tc.tile_pool(name="ps1", bufs=3, space="PSUM"))
        ps2_pool = es.enter_context(tc.tile_pool(name="ps2", bufs=5, space="PSUM"))

        # ---- input DMAs: need-first order, fanned across all 4 queues ----
        ctx_sb = [ctx_pool.tile([128, B * S], f16, name=f"ctx_{jc}")
                  for jc in range(2)]
        wt_sb = [wt_pool.tile([128, KV * C], f16, name=f"wt_{jc}")
                 for jc in range(2)]
        M2_sb = small_pool.tile([128, 2 * KV], f32, name="M2_sb")
        A_sb = small_pool.tile([128, 2 * B * KV], f32, name="A_sb")
        # need-first fan-out. Small shared deps (M2, A) go FIRST on their
        # queue: Tile coalesces DMA-completion waits by queue position, so a
        # consumer effectively waits for every DMA up to the one it needs.
        H = B * S // 2
        nc.sync.dma_start(wt_sb[0][:, 0:4 * C], wt2_d[0][:, 0:4 * C])
        nc.gpsimd.dma_start(wt_sb[1][:, 0:4 * C], wt2_d[1][:, 0:4 * C])
        nc.scalar.dma_start(M2_sb[:], M2_dram)
        nc.scalar.dma_start(A_sb[:], A_dram)
        nc.sync.dma_start(ctx_sb[0][:, 0:H], ctx2_d[0][:, 0:H])
        nc.gpsimd.dma_start(ctx_sb[1][:, 0:H], ctx2_d[1][:, 0:H])
        nc.scalar.dma_start(wt_sb[0][:, 4 * C:], wt2_d[0][:, 4 * C:])
        nc.scalar.dma_start(wt_sb[1][:, 4 * C:], wt2_d[1][:, 4 * C:])
        nc.sync.dma_start(ctx_sb[0][:, H:], ctx2_d[0][:, H:])
        nc.gpsimd.dma_start(ctx_sb[1][:, H:], ctx2_d[1][:, H:])

        # ---- warmup: ramp the PE p-state while inputs land; warm tanh table
        wsrc = small_pool.tile([128, 512], f16, name="wsrc")
        nc.vector.memset(wsrc[:], 0.0)
        wdst = small_pool.tile([128, 8], f16, name="wdst")
        # AP bias (not a float) so no const-tensor load lands on the sync
        # queue ahead of the input DMAs
        nc.scalar.activation(wdst[:, 0:1], wsrc[:, 0:1], TANH, bias=wsrc[:, 1:2])
        for _ in range(8):
            # fresh psum tile per matmul: WAW reuse semaphores would
            # otherwise fragment the p-state ramp
            wps = ps2_pool.tile([128, 512], f32, name="ps2")
            nc.tensor.matmul(wps[:], wsrc[:, 0:128], wsrc[:], start=True, stop=True)

        tmp2 = {}  # (pair, ch) -> tile [128, (h, kk, z)]
        for p in range(4):
            for ch in range(2):
                tmp2[p, ch] = tmp2_pool.tile([128, 2 * KV * S], f16,
                                             name=f"tmp2_{p}_{ch}")

        # Visit order: kk0-3 of a pair-duo first, then their kk4-7, so the
        # first 8 blocks depend only on the first wt DMA slice. Chunk g's
        # producer blocks are exactly 2g and 2g+1 under this mapping.
        SEG_PAIR = [0, 1, 0, 1, 2, 3, 2, 3]
        SEG_KK = [0, 0, 4, 4, 0, 0, 4, 4]

        def p1_block(j):
            """Phase-1 block j (j=0..31): 4 MMs, 2 drains. The first two
            blocks drain in batch-half slices, h=0 first: chunk 0's b0
            banks only need the h=0 halves, so they unlock earlier."""
            p = SEG_PAIR[j // 4]
            kk = SEG_KK[j // 4] + j % 4
            pss = {}
            for ch in range(2):
                ps = pss[ch] = ps1_pool.tile([128, 2 * S], f32, name="ps1")
                for jc in range(2):
                    lhsT = wt_sb[jc][:, kk * C + ch * 128: kk * C + ch * 128 + 128]
                    nc.tensor.matmul(ps[:], lhsT,
                                     ctx_sb[jc][:, 2 * p * S:(2 * p + 2) * S],
                                     start=(jc == 0), stop=(jc == 1))
                if j >= 2:
                    dst = tmp2[p, ch][:].rearrange("q (h k z) -> q h k z",
                                                   h=2, k=KV)[:, :, kk, :]
                    src = ps[:].rearrange("q (h z) -> q h z", h=2)
                    nc.vector.tensor_scalar_add(dst, src,
                                                M2_sb[:, ch * KV + kk:
                                                      ch * KV + kk + 1])
            if j < 2:
                for h in range(2):
                    for ch in range(2):
                        dst = tmp2[p, ch][:].rearrange(
                            "q (h k z) -> q h k z", h=2, k=KV)[:, h, kk, :]
                        nc.vector.tensor_scalar_add(
                            dst, pss[ch][:, h * S:(h + 1) * S],
                            M2_sb[:, ch * KV + kk: ch * KV + kk + 1])

        def p2_chunk(g, last=False):
            """Phase-2 chunk g (g=0..15): covers one (pair, kk-pair) for the
            pair's 2 batches: 16 MMs, 8 ACT tanh(+A bias), out DMAs."""
            pair = SEG_PAIR[g // 2]
            kkp = SEG_KK[g // 2] // 2 + g % 2
            for b in (2 * pair, 2 * pair + 1):
                eng = nc.sync if b % 2 == 0 else nc.gpsimd
                ot = ot_pool.tile([128, 4 * S], f16, name="ot")
                otv = ot[:].rearrange("p (k c s) -> p k c s", k=2, c=2)
                for zc in range(2):
                    bank = ps2_pool.tile([128, 512], f32, name="ps2")
                    for kx in range(2):
                        kk = 2 * kkp + kx
                        for st in range(2):
                            lhsT = tmp2[b // 2, st][:].rearrange(
                                "q (h k z) -> q h k z", h=2, k=KV)[
                                :, b % 2, kk, zc * 128:zc * 128 + 128]
                            nc.tensor.matmul(
                                bank[:, kx * 256:kx * 256 + 256], lhsT,
                                ctx_sb[st][:, b * S:(b + 1) * S],
                                start=(st == 0), stop=(st == 1))
                        nc.scalar.activation(
                            otv[:, kx, zc, :],
                            bank[:, kx * 256:kx * 256 + 256], TANH,
                            bias=A_sb[:, zc * 64 + b * KV + kk:
                                      zc * 64 + b * KV + kk + 1])
                    if last:  # split the final DMAs for a shorter tail
                        eng.dma_start(
                            out_d[b, 2 * kkp:2 * kkp + 2, zc].rearrange(
                                "k p s -> p k s"),
                            otv[:, :, zc, :])
                if not last:
                    eng.dma_start(
                        out_d[b, 2 * kkp:2 * kkp + 2].rearrange(
                            "k c p s -> p k c s"),
                        otv[:])

        # ---- flat pipeline: 32 pair-blocks, 16 chunks. Chunk g is emitted
        # immediately after its producer blocks 2g, 2g+1: only their 4
        # drains precede it, so the coalesced drain-wait threshold stays
        # exact and the first chunk starts as soon as B0/B1 are drained ----
        p1_block(0)
        p1_block(1)
        for g in range(16):
            p2_chunk(g, last=(g >= 14))
            if 2 * g + 2 < 32:
                p1_block(2 * g + 2)
            if 2 * g + 3 < 32:
                p1_block(2 * g + 3)

    nc.compile()
    return nc


def _install_profile_hook():
    """Register the NTFF profile hook that the image's boot skipped
    (antenv.axon_hooks shim is missing in this container)."""
    import sys as _sys
    import types as _types
    try:
        import antenv
        if "antenv.axon_hooks" not in _sys.modules:
            m = _types.ModuleType("antenv.axon_hooks")
            _h = [None]
            m.set_axon_ntff_profile_hook = lambda h: _h.__setitem__(0, h)
            m.get_axon_ntff_profile_hook = lambda: _h[0]
            _sys.modules["antenv.axon_hooks"] = m
            antenv.axon_hooks = m
        from antenv.axon_hooks import set_axon_ntff_profile_hook, get_axon_ntff_profile_hook
        if get_axon_ntff_profile_hook() is None:
            from trn_agent_boot.trn_boot import _ntff_profile_via_ctypes
            set_axon_ntff_profile_hook(_ntff_profile_via_ctypes("/opt/axon/libaxon_pjrt.so"))
    except Exception:
        pass


def run(inputs, trace=False, repeats=1):
    """Returns (full_output, BassKernelResults)."""
    from concourse.bass_utils import run_bass_kernel_spmd

    if trace:
        _install_profile_hook()
    per_core = _host_prep(**inputs)
    nc = _build_program()
    import os as _os
    _tc = [int(x) for x in _os.environ.get("KERNEL_TRACE_CORES", "0").split(",")]
    times = []
    res = None
    for _ in range(repeats):
        res = run_bass_kernel_spmd(nc, per_core, list(range(NCORES)), trace=trace,
                                   trace_cores=_tc if trace else None)
        if res.exec_time_ns is not None:
            times.append(res.exec_time_ns)
    if times:
        res.all_exec_times_ns = times
    # per-core scratch is (B, KV, 2, 128, S): reshape to (B, KV, Z, S) then
    # transpose to (B, S, Z, KV); concat k across cores
    out = np.concatenate(
        [res.results[c]["out"].astype(np.float32)
         .reshape(B, KV, S, S).transpose(0, 3, 2, 1)
         for c in range(NCORES)], axis=3)
    out = np.ascontiguousarray(out)
    return out, res


def kernel(**inputs) -> np.ndarray:
    out, _ = run(inputs, trace=False)
    return out
